# revision 74
# baseline (speedup 1.0000x reference)
"""Trainium2 Bass kernel for GQA attention (dense_transformer).

Sharding (8 cores): core c = (batch b = c//4, tp-rank g = c%4).
Each core computes q/k/v projections + RoPE + causal attention + partial
output projection for heads {g, g+4, g+8, g+12} (which all map to kv head
g under the reference's jnp.tile repeat), then a 4-way ReduceScatter over
the batch group combines the output projection partials; each core ends
up with a disjoint quarter of each 512-row i-chunk of the output.

Layout choices:
  - All matmul operands are bf16 (fp32 PSUM accumulate): same PE rate as
    float32r at 512-col moving tiles, but halves DMA/SBUF traffic and gets
    fast weight loads (FWL) on hardware.
  - Scores are computed transposed (scores^T[j, i]) so softmax probs feed
    the P@V matmul directly as the moving operand; the softmax denominator
    comes from an extra all-ones stationary matmul into a separate PSUM
    bank (partition-broadcast for free).
  - No max-subtraction in softmax: scores are O(6) sigma for this problem
    scale, exp() stays well inside fp32 range.  The additive mask is
    applied as exp(mask) multiplicative block patterns; all-zero blocks
    are skipped entirely (causality), all-pass blocks skip the multiply.
  - x is transposed on host (d-major) so every matmul contracts along
    partitions without any on-device transposes (v needs 128x128 PE
    transposes only).
  - RoPE stages PSUM->SBUF bf16 via the scalar engine so every DVE operand
    is 2-byte + SBUF (4x DVE rate); out-proj PSUM->SBUF copies run on DVE
    to keep the scalar engine free for the exp chain.
  - Attention is software-pipelined 2 deep (scores jt+1, jt+2 issue before
    av/den jt) to hide exp + mask-mul latency.
  - Single fused i-chunk pipeline: proj(qc) -> rope -> attention(qc) ->
    out-proj(qc) -> ReduceScatter(qc), all sharing one 8-bank PSUM pool,
    so collectives and DMA overlap compute of the next chunk.
"""

import sys

sys.path.insert(0, "/opt/trn_rl_repo")

import numpy as np

# ---------------------------------------------------------------- constants
B = 2
S = 2048
DIM = 2048
NH = 16
NKV = 4
HD = 128          # head dim == partition count
P = 128
CH = 512          # i-chunk columns (moving dim)
NQC = S // CH     # 4 i-chunks
NJT = S // P      # 16 j-tiles
DT = DIM // P     # 16 d-tiles (contraction)
HPC = NH // 4     # 4 heads per core
NOC = DIM // CH   # 4 output column chunks
N_CORES = 8
GROUPS = [[0, 1, 2, 3], [4, 5, 6, 7]]
SCALE = 1.0 / float(np.sqrt(HD))

_prog_cache: dict = {}


def _mask_schedule(mask):
    """Per (i-chunk, j-tile) block status from the additive mask.

    Returns (sched, patterns): sched[qc] = tuple of (jt, pat_idx|None) for
    non-skipped j-tiles; patterns = list of [P, CH] f32 multiplicative
    exp(mask) blocks (transposed to [j, i] layout).
    """
    m = np.asarray(mask, dtype=np.float32).reshape(S, S)
    pats = {}
    plist = []
    sched = []
    for qc in range(NQC):
        row = []
        for jt in range(NJT):
            blk = m[qc * CH:(qc + 1) * CH, jt * P:(jt + 1) * P]
            em = np.exp(blk.astype(np.float64)).astype(np.float32)
            if np.all(em == 1.0):
                row.append((jt, None, 0))
            elif np.all(em == 0.0):
                continue
            else:
                pt = np.ascontiguousarray(em.T)  # [j=128, i=512]
                key = pt.tobytes()
                if key not in pats:
                    pats[key] = len(plist)
                    plist.append(pt)
                # leading fully-masked i-columns can be cropped from the
                # moving dim (bf16 matmuls run full rate at any width)
                zc = 0
                while zc < CH and not pt[:, zc].any():
                    zc += 1
                row.append((jt, pats[key], min(zc - zc % P, CH - P)))
        if not row:
            raise ValueError(
                f"i-chunk {qc} is fully masked; softmax would be undefined"
            )
        sched.append(tuple(row))
    return tuple(sched), plist


def _build_program(sched, n_pat):
    import concourse.bacc as bacc
    import concourse.mybir as mybir
    import concourse.tile as tile

    F32 = mybir.dt.float32
    F32R = mybir.dt.float32r
    BF16 = mybir.dt.bfloat16
    FP16 = mybir.dt.float16
    AF = mybir.ActivationFunctionType

    # does any chunk's attention read k/v from a later chunk (non-causal
    # mask)?  If so the fused one-pass pipeline is invalid: fall back to
    # two phases (all projections, then attention).
    future = any(
        jt >= (qc + 1) * (CH // P)
        for qc, row in enumerate(sched)
        for (jt, _pidx, _cr) in row
    )

    nc = bacc.Bacc(None, target_bir_lowering=False, num_devices=N_CORES)

    xt = nc.declare_dram_parameter("xt", [DIM, S], BF16, isOutput=False)
    wqs = nc.declare_dram_parameter("wqs", [DIM, HPC * HD], BF16, isOutput=False)
    wks = nc.declare_dram_parameter("wks", [DIM, HD], BF16, isOutput=False)
    wvs = nc.declare_dram_parameter("wvs", [DIM, HD], BF16, isOutput=False)
    wos = nc.declare_dram_parameter("wos", [HPC * HD, DIM], BF16, isOutput=False)
    cosf = nc.declare_dram_parameter("cosf", [HD, S], BF16, isOutput=False)
    sinf = nc.declare_dram_parameter("sinf", [HD, S], BF16, isOutput=False)
    onesm = nc.declare_dram_parameter("onesm", [P, P], BF16, isOutput=False)
    ident = nc.declare_dram_parameter("ident", [P, P], BF16, isOutput=False)
    dpat = nc.declare_dram_parameter("dpat", [max(n_pat, 1), P, CH], BF16, isOutput=False)
    out = nc.declare_dram_parameter("out", [NJT, P // 4, DIM], FP16, isOutput=True)

    o_part = nc.dram_tensor("o_part", [S, DIM], FP16)
    rs_out = nc.dram_tensor("rs_out", [NJT, P // 4, DIM], FP16)

    shuffle_mask = [i ^ 1 for i in range(32)]

    with tile.TileContext(nc) as tc:
        with (
            tc.tile_pool(name="const", bufs=1) as constp,
            tc.tile_pool(name="wq_sb", bufs=1) as wqp,
            tc.tile_pool(name="wkv_sb", bufs=2) as wkvp,
            tc.tile_pool(name="wo_sb", bufs=HPC) as wop,
            tc.tile_pool(name="kT", bufs=1) as kTp,
            tc.tile_pool(name="vsb", bufs=NJT) as vsbp,
            tc.tile_pool(name="qT", bufs=(NQC if future else 2) * HPC) as qTp,
            tc.tile_pool(name="ohT", bufs=(1 if future else 2) * HPC) as ohTp,
            tc.tile_pool(name="xtp", bufs=5) as xtp,
            tc.tile_pool(name="vt_sb", bufs=3) as vtsbp,
            tc.tile_pool(name="probs", bufs=6) as probsp,
            tc.tile_pool(name="tmp", bufs=6) as tmpp,
            tc.tile_pool(name="o_sb", bufs=3) as osbp,
            tc.tile_pool(name="rsbp", bufs=2) as rsbp,
            tc.tile_pool(name="ps", bufs=6, space="PSUM") as psp,
            tc.tile_pool(name="ps_ad", bufs=2, space="PSUM") as psadp,
        ):
            # ---- constants + resident weights -------------------------------
            cos_sb = constp.tile([HD, S], BF16, tag="cos")
            sin_sb = constp.tile([HD, S], BF16, tag="sin")
            ones_sb = constp.tile([P, P], BF16, tag="ones")
            id_sb = constp.tile([P, P], BF16, tag="ident")
            dpat_sb = constp.tile([P, max(n_pat, 1) * CH], BF16, tag="dpat")

            # wq lives in ONE tile laid out [p, (t, h, m)] so a whole
            # d-quarter (all 4 heads) loads with a single DMA instruction —
            # the SP queue's ~790ns per-issue cost was starving the ramp
            DQ = DT // 4
            wq_all = wqp.tile([P, DT * HPC * HD], BF16, tag="wq", name="wq_all")

            def wq_slice(d, hl):
                o = (d * HPC + hl) * HD
                return wq_all[:, o:o + HD]

            wk_sb = wkvp.tile([P, DT * HD], BF16, tag="wkv")
            wv_sb = wkvp.tile([P, DT * HD], BF16, tag="wkv")
            wo_sb = [
                wop.tile([HD, DIM], BF16, tag="wo", name=f"wo{hl}")
                for hl in range(HPC)
            ]

            # xt is loaded one d-quarter (4 tiles = [P, 4*CH]) per DMA
            pre_xt = []

            def xt_qload(qc, db, warm=False):
                t = xtp.tile([P, DQ * CH], BF16, tag="xt", name=f"xtq{qc}_{db}")
                nc.sync.dma_start(
                    t[:].rearrange("p (t c) -> p t c", c=CH),
                    xt[db * DQ * P:(db + 1) * DQ * P,
                       qc * CH:(qc + 1) * CH].rearrange("(t p) c -> p t c", p=P),
                )
                if warm:
                    pre_xt.append(t)
                return t

            def wkv_quarter(db):
                dbs = slice(db * DQ * HD, (db + 1) * DQ * HD)
                rows = slice(db * DQ * P, (db + 1) * DQ * P)
                nc.sync.dma_start(
                    wk_sb[:, dbs].rearrange("p (t m) -> p t m", m=HD),
                    wks[rows, :].rearrange("(t p) m -> p t m", p=P),
                )
                nc.sync.dma_start(
                    wv_sb[:, dbs].rearrange("p (t m) -> p t m", m=HD),
                    wvs[rows, :].rearrange("(t p) m -> p t m", p=P),
                )

            def wq_quarter(db):
                o = db * DQ * HPC * HD
                rows = slice(db * DQ * P, (db + 1) * DQ * P)
                nc.sync.dma_start(
                    wq_all[:, o:o + DQ * HPC * HD].rearrange(
                        "p (t h m) -> p t h m", h=HPC, m=HD),
                    wqs[rows, :].rearrange(
                        "(t p) (h m) -> p t h m", p=P, m=HD),
                )

            # HAM warm-up: the PE clock-gate needs ~3.4us of sustained
            # activity to lift 1.2 -> 2.4 GHz, and every NEFF execution
            # starts cold.  Fill the initial DMA-wait window with throwaway
            # matmuls (results never read; psadp slot is not reused until
            # attention, so this stalls nothing).
            warm_in = tmpp.tile([P, P], BF16, tag="tmp", name="warm_in")
            nc.vector.memset(warm_in[:], 0.0)
            warm_ps = psadp.tile([P, P], F32, tag="ps_ad", name="warm_ps")
            for _ in range(32):
                nc.tensor.matmul(warm_ps[:], warm_in[:], warm_in[:],
                                 start=True, stop=True)

            wkv_quarter(0)
            xt_qload(0, 0, warm=True)
            wq_quarter(0)
            wkv_quarter(1)
            xt_qload(0, 1, warm=True)
            wq_quarter(1)
            wkv_quarter(2)
            xt_qload(0, 2, warm=True)
            wq_quarter(2)
            wkv_quarter(3)
            xt_qload(0, 3, warm=True)
            wq_quarter(3)

            kT = kTp.tile([HD, S], BF16, tag="kT")
            vsb = [vsbp.tile([P, HD], BF16, tag="vsb", name=f"vsb{i}") for i in range(NJT)]

            def emit_oproj(qc, ohT, split_rs=False):
                # output projection for chunk qc's rows + ReduceScatter.
                # i-tile-outer so each row block can ReduceScatter as soon as
                # it is done (used for the last chunk to hide the tail).
                nit = CH // P
                for it in range(nit):
                    ti = qc * nit + it
                    isl = slice(ti * P, (ti + 1) * P)
                    ob = osbp.tile([P, DIM], FP16, tag="ob", name=f"ob{qc}_{it}")
                    for oc in range(NOC):
                        osl = slice(oc * CH, (oc + 1) * CH)
                        ps_o = psp.tile([P, CH], F32, tag="ps", name=f"pso{qc}_{it}_{oc}")
                        for hl in range(HPC):
                            nc.tensor.matmul(
                                ps_o[:],
                                ohT[hl][:, it * P:(it + 1) * P],
                                wo_sb[hl][:, osl],
                                start=(hl == 0), stop=(hl == HPC - 1),
                            )
                        # ACT copy: the scalar engine has slack in every chunk
                        # window, while DVE saturates during attention (mask
                        # muls + inv/normalize) and would delay PSUM release
                        nc.scalar.activation(ob[:, osl], ps_o[:], AF.Copy)
                    nc.sync.dma_start(o_part[isl, :], ob[:])
                nc.gpsimd.collective_compute(
                    "ReduceScatter",
                    mybir.AluOpType.add,
                    replica_groups=GROUPS,
                    ins=[o_part[qc * CH:(qc + 1) * CH, :]],
                    outs=[rs_out[qc * nit:(qc + 1) * nit]],
                )
                rsb = rsbp.tile([P, DIM], FP16, tag="rsb", name=f"rsb{qc}")
                nc.sync.dma_start(
                    rsb[:],
                    rs_out[qc * nit:(qc + 1) * nit].rearrange("a b c -> (a b) c"))
                nc.sync.dma_start(
                    out[qc * nit:(qc + 1) * nit].rearrange("a b c -> (a b) c"),
                    rsb[:])

            def emit_attn(qc, qT):
                # attention for all heads on this chunk
                acts = sched[qc]
                nact = len(acts)
                ohT = []
                for hl in range(HPC):
                    # separate pool: av/den drain late (via the DVE normalize)
                    # and must not sit in the fast-cycling pool's rotation
                    ps_av = psadp.tile([HD, CH], F32, tag="ps_ad", name=f"psav{qc}_{hl}")
                    ps_den = psadp.tile([P, CH], F32, tag="ps_ad", name=f"psden{qc}_{hl}")

                    def emit_avden(jt, pr, cr, idx):
                        nc.tensor.matmul(
                            ps_av[:, cr:], vsb[jt][:], pr[:, cr:],
                            start=(idx == 0), stop=(idx == nact - 1),
                        )
                        nc.tensor.matmul(
                            ps_den[:, cr:], ones_sb[:], pr[:, cr:],
                            start=(idx == 0), stop=(idx == nact - 1),
                        )

                    # software-pipelined 2 deep: scores(jt+1), scores(jt+2)
                    # issue before av/den(jt) so the exp (+ mask-mul) latency
                    # hides behind the next two scores matmuls
                    pending = []
                    for idx, (jt, pidx, crop) in enumerate(acts):
                        cr = crop if idx > 0 else 0
                        ps_s = psp.tile([P, CH], F32, tag="ps", name=f"pss{qc}_{hl}_{jt}")
                        nc.tensor.matmul(
                            ps_s[:, cr:],
                            kT[:, jt * P:(jt + 1) * P],
                            qT[hl][:, cr:],
                            start=True, stop=True,
                        )
                        pr = probsp.tile([P, CH], BF16, tag="pr", name=f"pr{qc}")
                        nc.scalar.activation(pr[:, cr:], ps_s[:, cr:], AF.Exp, scale=SCALE)
                        if pidx is not None:
                            nc.vector.tensor_mul(
                                pr[:, cr:], pr[:, cr:],
                                dpat_sb[:, pidx * CH + cr:(pidx + 1) * CH],
                            )
                        pending.append((jt, pr, cr, idx))
                        # early chunks have few j-tiles and are latency-bound:
                        # pipeline deeper there
                        if len(pending) > (3 if nact <= 8 else 2):
                            emit_avden(*pending.pop(0))
                    for args in pending:
                        emit_avden(*args)
                    inv = tmpp.tile([P, CH], F32, tag="tmp", name=f"inv{qc}")
                    nc.vector.reciprocal(inv[:], ps_den[:])
                    oh = ohTp.tile([HD, CH], BF16, tag="ohT", name=f"ohT{qc}_{hl}")
                    nc.vector.tensor_mul(oh[:], ps_av[:], inv[:])
                    ohT.append(oh)
                return ohT

            # ---- fused per-chunk pipeline -----------------------------------
            prev_ohT = None
            saved_qT = []
            for qc in range(NQC):
                csl = slice(qc * CH, (qc + 1) * CH)

                # projections: accumulate q (4 heads), k, v over d-tiles
                ps_q = [psp.tile([P, CH], F32, tag="ps", name=f"psq{qc}_{i}") for i in range(HPC)]
                ps_k = psp.tile([P, CH], F32, tag="ps", name=f"psk{qc}")
                ps_v = psp.tile([P, CH], F32, tag="ps", name=f"psv{qc}")
                # chunk 0 is DMA-gated: interleave per d-quarter so matmuls
                # track arriving weight/x quarters.  chunks 1-3 are PSUM-
                # gated at the boundary: run ALL k/v matmuls first so the 4
                # ps_q banks are not needed until ~7us into the chunk,
                # absorbing the previous chunk's attention-bank drain.
                xqs = []
                for db in range(4):
                    if qc == 0 and db < len(pre_xt):
                        xqs.append(pre_xt[db])
                    else:
                        xqs.append(xt_qload(qc, db))

                def xview(d):
                    db = d // DQ
                    return xqs[db][:, (d - db * DQ) * CH:(d - db * DQ + 1) * CH]

                def kv_d(d):
                    dsl = slice(d * HD, (d + 1) * HD)
                    st, sp = (d == 0), (d == DT - 1)
                    nc.tensor.matmul(ps_k[:], wk_sb[:, dsl], xview(d), start=st, stop=sp)
                    nc.tensor.matmul(ps_v[:], wv_sb[:, dsl], xview(d), start=st, stop=sp)

                def q_d(d):
                    st, sp = (d == 0), (d == DT - 1)
                    for hl in range(HPC):
                        nc.tensor.matmul(
                            ps_q[hl][:], wq_slice(d, hl), xview(d),
                            start=st, stop=sp,
                        )

                if qc == 0:
                    for db in range(4):
                        for d in range(db * DQ, (db + 1) * DQ):
                            kv_d(d)
                        for d in range(db * DQ, (db + 1) * DQ):
                            q_d(d)
                else:
                    for d in range(DT):
                        kv_d(d)
                    for d in range(DT):
                        q_d(d)

                # per-chunk slices of the RoPE tables: chunk 0's rope only
                # needs the first 512 columns, so don't front-load 4MB
                # constants and next-phase weights go on the scalar-engine
                # HWDGE queue so the SP queue stays dedicated to xt tiles
                nc.scalar.dma_start(cos_sb[:, csl], cosf[:, csl])
                nc.scalar.dma_start(sin_sb[:, csl], sinf[:, csl])
                if qc == 0:
                    nc.scalar.dma_start(ones_sb[:], onesm[:])
                    nc.scalar.dma_start(id_sb[:], ident[:])
                    nc.scalar.dma_start(
                        dpat_sb[:].rearrange("p (n c) -> p n c", c=CH),
                        dpat[:].rearrange("n p c -> p n c"),
                    )
                    for hl in range(HPC):
                        nc.scalar.dma_start(
                            wo_sb[hl][:], wos[hl * HD:(hl + 1) * HD, :],
                        )

                # v first: its PSUM->SBUF copy + transposes + vsb copies gate
                # the attention AV matmuls, and must not queue behind the
                # rope's scalar-engine staging copies
                vt = vtsbp.tile([P, CH], BF16, tag="vt", name=f"vt{qc}")
                nc.scalar.activation(vt[:], ps_v[:], AF.Copy)
                for jl in range(CH // P):
                    tps = psp.tile([P, P], BF16, tag="ps", name=f"tps{qc}_{jl}")
                    nc.tensor.transpose(tps[:], vt[:, jl * P:(jl + 1) * P], id_sb[:])
                    nc.scalar.activation(vsb[qc * (CH // P) + jl][:], tps[:], AF.Copy)

                # RoPE q heads -> per-chunk qT tiles; k -> resident kT
                qT = []
                for hl in range(HPC):
                    dst = qTp.tile([HD, CH], BF16, tag="qT", name=f"qT{qc}_{hl}")
                    qT.append(dst)
                # q heads first: attention on this chunk is gated by qT (old kT
                # columns are already rope'd); the new kT columns are only
                # needed at the diagonal j-tiles, late in the j-loop.
                rope_jobs = [(ps_q[hl], qT[hl][:]) for hl in range(HPC)]
                rope_jobs.append((ps_k, kT[:, csl]))
                if qc == 0:
                    rope_jobs.insert(0, rope_jobs.pop())
                for src, dst in rope_jobs:
                    # stage PSUM f32 -> SBUF bf16 on the scalar engine so
                    # every DVE operand is 2-byte + SBUF (4x DVE rate)
                    qf = tmpp.tile([P, CH], BF16, tag="tmp", name=f"qf{qc}")
                    nc.scalar.activation(qf[:], src[:], AF.Copy)
                    swp = tmpp.tile([P, CH], BF16, tag="tmp", name=f"sw{qc}")
                    nc.vector.stream_shuffle(swp[:], qf[:], shuffle_mask)
                    tcos = tmpp.tile([P, CH], BF16, tag="tmp", name=f"tc{qc}")
                    nc.vector.tensor_mul(tcos[:], qf[:], cos_sb[:, csl])
                    tsin = tmpp.tile([P, CH], BF16, tag="tmp", name=f"ts{qc}")
                    nc.vector.tensor_mul(tsin[:], swp[:], sin_sb[:, csl])
                    nc.vector.tensor_add(dst, tcos[:], tsin[:])

                if future:
                    saved_qT.append(qT)
                    continue

                # previous chunk's output projection: PE work that overlaps the
                # RoPE (DVE) of this chunk
                if prev_ohT is not None:
                    emit_oproj(qc - 1, prev_ohT)
                prev_ohT = emit_attn(qc, qT)

            if future:
                for qc in range(NQC):
                    emit_oproj(qc, emit_attn(qc, saved_qT[qc]))
            else:
                emit_oproj(NQC - 1, prev_ohT)

    nc.finalize()
    return nc


def _get_program(sched, n_pat):
    key = (sched, n_pat)
    if key not in _prog_cache:
        _prog_cache[key] = _build_program(sched, n_pat)
    return _prog_cache[key]


def kernel(x, wq, wk, wv, wo, freqs_cos, freqs_sin, mask, start_pos=0, **_kw):
    from concourse.bass_utils import run_bass_kernel_spmd
    import ml_dtypes

    BF = ml_dtypes.bfloat16
    x = np.asarray(x, dtype=np.float32)
    wq = np.asarray(wq, dtype=np.float32)
    wk = np.asarray(wk, dtype=np.float32)
    wv = np.asarray(wv, dtype=np.float32)
    wo = np.asarray(wo, dtype=np.float32)
    fc = np.asarray(freqs_cos, dtype=np.float32)
    fs = np.asarray(freqs_sin, dtype=np.float32)

    sched, plist = _mask_schedule(mask)
    nc = _get_program(sched, len(plist))

    # RoPE tables expanded to head-dim channels (sin sign-interleaved so the
    # pair-swap shuffle needs no negation).
    cosf = np.repeat(fc.T, 2, axis=0)                            # [HD, S]
    sinf = np.repeat(fs.T, 2, axis=0)
    sinf[0::2, :] *= -1.0
    cosf = np.ascontiguousarray(cosf.astype(BF))
    sinf = np.ascontiguousarray(sinf.astype(BF))

    onesm = np.ones((P, P), dtype=BF)
    ident = np.eye(P, dtype=BF)
    dpat_arr = (
        np.stack(plist, axis=0).astype(BF)
        if plist
        else np.zeros((1, P, CH), dtype=BF)
    )

    xtb = [np.ascontiguousarray(x[b].T.astype(BF)) for b in range(B)]
    wq_bf = wq.astype(BF)
    wk_bf = wk.astype(BF)
    wv_bf = wv.astype(BF)
    in_maps = []
    for c in range(N_CORES):
        b, g = divmod(c, 4)
        hcols = np.concatenate(
            [np.arange(h * HD, (h + 1) * HD) for h in (g, g + 4, g + 8, g + 12)]
        )
        in_maps.append(
            dict(
                xt=xtb[b],
                wqs=np.ascontiguousarray(wq_bf[:, hcols]),
                wks=np.ascontiguousarray(wk_bf[:, g * HD:(g + 1) * HD]),
                wvs=np.ascontiguousarray(wv_bf[:, g * HD:(g + 1) * HD]),
                wos=np.ascontiguousarray(wo[hcols, :].astype(BF)),
                cosf=cosf,
                sinf=sinf,
                onesm=onesm,
                ident=ident,
                dpat=dpat_arr,
            )
        )

    res = run_bass_kernel_spmd(nc, in_maps, list(range(N_CORES)))

    out_full = np.empty((B, S, DIM), dtype=np.float32)
    nit = CH // P
    for c in range(N_CORES):
        b, g = divmod(c, 4)
        o = np.asarray(res.results[c]["out"], dtype=np.float32)
        for qc in range(NQC):
            r0 = qc * CH + g * P
            out_full[b, r0:r0 + P, :] = o[qc * nit:(qc + 1) * nit].reshape(P, DIM)
    return out_full



# revision 75
# speedup vs baseline: 1.0772x; 1.0772x over previous
"""Trainium2 Bass kernel for GQA attention (dense_transformer).

Sharding (8 cores): core c = (batch b = c//4, tp-rank g = c%4).
Each core computes q/k/v projections + RoPE + causal attention + partial
output projection for heads {g, g+4, g+8, g+12} (which all map to kv head
g under the reference's jnp.tile repeat), then a 4-way ReduceScatter over
the batch group combines the output projection partials; each core ends
up with a disjoint quarter of each 512-row i-chunk of the output.

Layout choices:
  - All matmul operands are bf16 (fp32 PSUM accumulate): same PE rate as
    float32r at 512-col moving tiles, but halves DMA/SBUF traffic and gets
    fast weight loads (FWL) on hardware.
  - Scores are computed transposed (scores^T[j, i]) so softmax probs feed
    the P@V matmul directly as the moving operand; the softmax denominator
    comes from an extra all-ones stationary matmul into a separate PSUM
    bank (partition-broadcast for free).
  - No max-subtraction in softmax: scores are O(6) sigma for this problem
    scale, exp() stays well inside fp32 range.  The additive mask is
    applied as exp(mask) multiplicative block patterns; all-zero blocks
    are skipped entirely (causality), all-pass blocks skip the multiply.
  - x is transposed on host (d-major) so every matmul contracts along
    partitions without any on-device transposes (v needs 128x128 PE
    transposes only).
  - RoPE stages PSUM->SBUF bf16 via the scalar engine so every DVE operand
    is 2-byte + SBUF (2x DVE rate).
  - Weight/x DMAs are batched (wq is one [p,(d,h,m)]-layout tile loaded a
    d-quarter per DMA; x a 4-tile quarter per DMA): the SP queue's ~790ns
    per-issue cost otherwise starves the start-up ramp.
  - Attention is software-pipelined 2 deep (3 deep on early latency-bound
    chunks) to hide exp + mask-mul latency.
  - PSUM is split 6+2: ps_av/ps_den drain late (through the DVE
    normalize) and live in their own pool so the round-robin rotation of
    the fast-cycling pool never blocks on them.
  - A 32-matmul warm-up preamble lifts the PE HAM clock gate (1.2->2.4
    GHz) during the DMA-bound ramp; results are never read.
  - Single fused i-chunk pipeline: proj(qc) -> rope -> attention(qc) ->
    out-proj(qc) -> ReduceScatter(qc); collectives and DMA overlap
    compute of the next chunk.
"""

import sys

sys.path.insert(0, "/opt/trn_rl_repo")

import numpy as np

# ---------------------------------------------------------------- constants
B = 2
S = 2048
DIM = 2048
NH = 16
NKV = 4
HD = 128          # head dim == partition count
P = 128
CH = 512          # i-chunk columns (moving dim)
NQC = S // CH     # 4 i-chunks
NJT = S // P      # 16 j-tiles
DT = DIM // P     # 16 d-tiles (contraction)
HPC = NH // 4     # 4 heads per core
NOC = DIM // CH   # 4 output column chunks
N_CORES = 8
GROUPS = [[0, 1, 2, 3], [4, 5, 6, 7]]
SCALE = 1.0 / float(np.sqrt(HD))

_prog_cache: dict = {}


def _mask_schedule(mask):
    """Per (i-chunk, j-tile) block status from the additive mask.

    Returns (sched, patterns): sched[qc] = tuple of (jt, pat_idx|None) for
    non-skipped j-tiles; patterns = list of [P, CH] f32 multiplicative
    exp(mask) blocks (transposed to [j, i] layout).
    """
    m = np.asarray(mask, dtype=np.float32).reshape(S, S)
    pats = {}
    plist = []
    sched = []
    for qc in range(NQC):
        row = []
        for jt in range(NJT):
            blk = m[qc * CH:(qc + 1) * CH, jt * P:(jt + 1) * P]
            em = np.exp(blk.astype(np.float64)).astype(np.float32)
            if np.all(em == 1.0):
                row.append((jt, None, 0))
            elif np.all(em == 0.0):
                continue
            else:
                pt = np.ascontiguousarray(em.T)  # [j=128, i=512]
                key = pt.tobytes()
                if key not in pats:
                    pats[key] = len(plist)
                    plist.append(pt)
                # leading fully-masked i-columns can be cropped from the
                # moving dim (bf16 matmuls run full rate at any width)
                zc = 0
                while zc < CH and not pt[:, zc].any():
                    zc += 1
                row.append((jt, pats[key], min(zc - zc % P, CH - P)))
        if not row:
            raise ValueError(
                f"i-chunk {qc} is fully masked; softmax would be undefined"
            )
        sched.append(tuple(row))
    return tuple(sched), plist


def _build_program(sched, n_pat):
    import concourse.bacc as bacc
    import concourse.mybir as mybir
    import concourse.tile as tile

    F32 = mybir.dt.float32
    F32R = mybir.dt.float32r
    BF16 = mybir.dt.bfloat16
    FP16 = mybir.dt.float16
    AF = mybir.ActivationFunctionType

    # does any chunk's attention read k/v from a later chunk (non-causal
    # mask)?  If so the fused one-pass pipeline is invalid: fall back to
    # two phases (all projections, then attention).
    future = any(
        jt >= (qc + 1) * (CH // P)
        for qc, row in enumerate(sched)
        for (jt, _pidx, _cr) in row
    )

    nc = bacc.Bacc(None, target_bir_lowering=False, num_devices=N_CORES)

    xt = nc.declare_dram_parameter("xt", [DIM, S], BF16, isOutput=False)
    wqs = nc.declare_dram_parameter("wqs", [DIM, HPC * HD], BF16, isOutput=False)
    wks = nc.declare_dram_parameter("wks", [DIM, HD], BF16, isOutput=False)
    wvs = nc.declare_dram_parameter("wvs", [DIM, HD], BF16, isOutput=False)
    wos = nc.declare_dram_parameter("wos", [HPC * HD, DIM], BF16, isOutput=False)
    cosf = nc.declare_dram_parameter("cosf", [HD, S], BF16, isOutput=False)
    sinf = nc.declare_dram_parameter("sinf", [HD, S], BF16, isOutput=False)
    onesm = nc.declare_dram_parameter("onesm", [P, P], BF16, isOutput=False)
    ident = nc.declare_dram_parameter("ident", [P, P], BF16, isOutput=False)
    dpat = nc.declare_dram_parameter("dpat", [max(n_pat, 1), P, CH], BF16, isOutput=False)
    out = nc.declare_dram_parameter("out", [NJT, P // 4, DIM], FP16, isOutput=True)

    o_part = nc.dram_tensor("o_part", [S, DIM], FP16)
    rs_out = nc.dram_tensor("rs_out", [NJT, P // 4, DIM], FP16)

    shuffle_mask = [i ^ 1 for i in range(32)]

    with tile.TileContext(nc) as tc:
        with (
            tc.tile_pool(name="const", bufs=1) as constp,
            tc.tile_pool(name="wq_sb", bufs=1) as wqp,
            tc.tile_pool(name="wkv_sb", bufs=2) as wkvp,
            tc.tile_pool(name="wo_sb", bufs=HPC) as wop,
            tc.tile_pool(name="kT", bufs=1) as kTp,
            tc.tile_pool(name="vsb", bufs=NJT) as vsbp,
            tc.tile_pool(name="qT", bufs=(NQC if future else 2) * HPC) as qTp,
            tc.tile_pool(name="ohT", bufs=(1 if future else 2) * HPC) as ohTp,
            tc.tile_pool(name="xtp", bufs=5) as xtp,
            tc.tile_pool(name="vt_sb", bufs=3) as vtsbp,
            tc.tile_pool(name="probs", bufs=6) as probsp,
            tc.tile_pool(name="tmp", bufs=6) as tmpp,
            tc.tile_pool(name="o_sb", bufs=3) as osbp,
            tc.tile_pool(name="rsbp", bufs=2) as rsbp,
            tc.tile_pool(name="ps", bufs=6, space="PSUM") as psp,
            tc.tile_pool(name="ps_ad", bufs=2, space="PSUM") as psadp,
        ):
            # ---- constants + resident weights -------------------------------
            cos_sb = constp.tile([HD, S], BF16, tag="cos")
            sin_sb = constp.tile([HD, S], BF16, tag="sin")
            ones_sb = constp.tile([P, P], BF16, tag="ones")
            id_sb = constp.tile([P, P], BF16, tag="ident")
            dpat_sb = constp.tile([P, max(n_pat, 1) * CH], BF16, tag="dpat")

            # wq lives in ONE tile laid out [p, (t, h, m)] so a whole
            # d-quarter (all 4 heads) loads with a single DMA instruction —
            # the SP queue's ~790ns per-issue cost was starving the ramp
            DQ = DT // 4
            wq_all = wqp.tile([P, DT * HPC * HD], BF16, tag="wq", name="wq_all")

            def wq_slice(d, hl):
                o = (d * HPC + hl) * HD
                return wq_all[:, o:o + HD]

            wk_sb = wkvp.tile([P, DT * HD], BF16, tag="wkv")
            wv_sb = wkvp.tile([P, DT * HD], BF16, tag="wkv")
            wo_sb = [
                wop.tile([HD, DIM], BF16, tag="wo", name=f"wo{hl}")
                for hl in range(HPC)
            ]

            # xt is loaded one d-quarter (4 tiles = [P, 4*CH]) per DMA
            pre_xt = []

            def xt_qload(qc, db, warm=False):
                t = xtp.tile([P, DQ * CH], BF16, tag="xt", name=f"xtq{qc}_{db}")
                nc.sync.dma_start(
                    t[:].rearrange("p (t c) -> p t c", c=CH),
                    xt[db * DQ * P:(db + 1) * DQ * P,
                       qc * CH:(qc + 1) * CH].rearrange("(t p) c -> p t c", p=P),
                )
                if warm:
                    pre_xt.append(t)
                return t

            def wkv_quarter(db):
                dbs = slice(db * DQ * HD, (db + 1) * DQ * HD)
                rows = slice(db * DQ * P, (db + 1) * DQ * P)
                nc.sync.dma_start(
                    wk_sb[:, dbs].rearrange("p (t m) -> p t m", m=HD),
                    wks[rows, :].rearrange("(t p) m -> p t m", p=P),
                )
                nc.sync.dma_start(
                    wv_sb[:, dbs].rearrange("p (t m) -> p t m", m=HD),
                    wvs[rows, :].rearrange("(t p) m -> p t m", p=P),
                )

            def wq_quarter(db):
                o = db * DQ * HPC * HD
                rows = slice(db * DQ * P, (db + 1) * DQ * P)
                nc.sync.dma_start(
                    wq_all[:, o:o + DQ * HPC * HD].rearrange(
                        "p (t h m) -> p t h m", h=HPC, m=HD),
                    wqs[rows, :].rearrange(
                        "(t p) (h m) -> p t h m", p=P, m=HD),
                )

            # HAM warm-up: the PE clock-gate needs ~3.4us of sustained
            # activity to lift 1.2 -> 2.4 GHz, and every NEFF execution
            # starts cold.  Fill the initial DMA-wait window with throwaway
            # matmuls (results never read; psadp slot is not reused until
            # attention, so this stalls nothing).
            warm_in = tmpp.tile([P, P], BF16, tag="tmp", name="warm_in")
            nc.vector.memset(warm_in[:], 0.0)
            warm_ps = psadp.tile([P, P], F32, tag="ps_ad", name="warm_ps")
            for _ in range(32):
                nc.tensor.matmul(warm_ps[:], warm_in[:], warm_in[:],
                                 start=True, stop=True)

            wkv_quarter(0)
            xt_qload(0, 0, warm=True)
            wq_quarter(0)
            wkv_quarter(1)
            xt_qload(0, 1, warm=True)
            wq_quarter(1)
            wkv_quarter(2)
            xt_qload(0, 2, warm=True)
            wq_quarter(2)
            wkv_quarter(3)
            xt_qload(0, 3, warm=True)
            wq_quarter(3)

            kT = kTp.tile([HD, S], BF16, tag="kT")
            vsb = [vsbp.tile([P, HD], BF16, tag="vsb", name=f"vsb{i}") for i in range(NJT)]

            def emit_oproj(qc, ohT, split_rs=False):
                # output projection for chunk qc's rows + ReduceScatter.
                # i-tile-outer so each row block can ReduceScatter as soon as
                # it is done (used for the last chunk to hide the tail).
                nit = CH // P
                for it in range(nit):
                    ti = qc * nit + it
                    isl = slice(ti * P, (ti + 1) * P)
                    ob = osbp.tile([P, DIM], FP16, tag="ob", name=f"ob{qc}_{it}")
                    for oc in range(NOC):
                        osl = slice(oc * CH, (oc + 1) * CH)
                        ps_o = psp.tile([P, CH], F32, tag="ps", name=f"pso{qc}_{it}_{oc}")
                        for hl in range(HPC):
                            nc.tensor.matmul(
                                ps_o[:],
                                ohT[hl][:, it * P:(it + 1) * P],
                                wo_sb[hl][:, osl],
                                start=(hl == 0), stop=(hl == HPC - 1),
                            )
                        # ACT copy: the scalar engine has slack in every chunk
                        # window, while DVE saturates during attention (mask
                        # muls + inv/normalize) and would delay PSUM release
                        nc.scalar.activation(ob[:, osl], ps_o[:], AF.Copy)
                    nc.sync.dma_start(o_part[isl, :], ob[:])
                nc.gpsimd.collective_compute(
                    "ReduceScatter",
                    mybir.AluOpType.add,
                    replica_groups=GROUPS,
                    ins=[o_part[qc * CH:(qc + 1) * CH, :]],
                    outs=[rs_out[qc * nit:(qc + 1) * nit]],
                )
                rsb = rsbp.tile([P, DIM], FP16, tag="rsb", name=f"rsb{qc}")
                nc.sync.dma_start(
                    rsb[:],
                    rs_out[qc * nit:(qc + 1) * nit].rearrange("a b c -> (a b) c"))
                nc.sync.dma_start(
                    out[qc * nit:(qc + 1) * nit].rearrange("a b c -> (a b) c"),
                    rsb[:])

            def emit_attn(qc, qT):
                # attention for all heads on this chunk
                acts = sched[qc]
                nact = len(acts)
                ohT = []
                for hl in range(HPC):
                    # separate pool: av/den drain late (via the DVE normalize)
                    # and must not sit in the fast-cycling pool's rotation
                    ps_av = psadp.tile([HD, CH], F32, tag="ps_ad", name=f"psav{qc}_{hl}")
                    ps_den = psadp.tile([P, CH], F32, tag="ps_ad", name=f"psden{qc}_{hl}")

                    def emit_avden(jt, pr, cr, idx):
                        nc.tensor.matmul(
                            ps_av[:, cr:], vsb[jt][:], pr[:, cr:],
                            start=(idx == 0), stop=(idx == nact - 1),
                        )
                        nc.tensor.matmul(
                            ps_den[:, cr:], ones_sb[:], pr[:, cr:],
                            start=(idx == 0), stop=(idx == nact - 1),
                        )

                    # software-pipelined 2 deep: scores(jt+1), scores(jt+2)
                    # issue before av/den(jt) so the exp (+ mask-mul) latency
                    # hides behind the next two scores matmuls
                    pending = []
                    for idx, (jt, pidx, crop) in enumerate(acts):
                        cr = crop if idx > 0 else 0
                        ps_s = psp.tile([P, CH], F32, tag="ps", name=f"pss{qc}_{hl}_{jt}")
                        nc.tensor.matmul(
                            ps_s[:, cr:],
                            kT[:, jt * P:(jt + 1) * P],
                            qT[hl][:, cr:],
                            start=True, stop=True,
                        )
                        pr = probsp.tile([P, CH], BF16, tag="pr", name=f"pr{qc}")
                        nc.scalar.activation(pr[:, cr:], ps_s[:, cr:], AF.Exp, scale=SCALE)
                        if pidx is not None:
                            nc.vector.tensor_mul(
                                pr[:, cr:], pr[:, cr:],
                                dpat_sb[:, pidx * CH + cr:(pidx + 1) * CH],
                            )
                        pending.append((jt, pr, cr, idx))
                        # early chunks have few j-tiles and are latency-bound:
                        # pipeline deeper there
                        if len(pending) > (3 if nact <= 8 else 2):
                            emit_avden(*pending.pop(0))
                    for args in pending:
                        emit_avden(*args)
                    inv = tmpp.tile([P, CH], F32, tag="tmp", name=f"inv{qc}")
                    nc.vector.reciprocal(inv[:], ps_den[:])
                    oh = ohTp.tile([HD, CH], BF16, tag="ohT", name=f"ohT{qc}_{hl}")
                    nc.vector.tensor_mul(oh[:], ps_av[:], inv[:])
                    ohT.append(oh)
                return ohT

            # ---- fused per-chunk pipeline -----------------------------------
            prev_ohT = None
            saved_qT = []
            for qc in range(NQC):
                csl = slice(qc * CH, (qc + 1) * CH)

                # projections: accumulate q (4 heads), k, v over d-tiles
                ps_q = [psp.tile([P, CH], F32, tag="ps", name=f"psq{qc}_{i}") for i in range(HPC)]
                ps_k = psp.tile([P, CH], F32, tag="ps", name=f"psk{qc}")
                ps_v = psp.tile([P, CH], F32, tag="ps", name=f"psv{qc}")
                # chunk 0 is DMA-gated: interleave per d-quarter so matmuls
                # track arriving weight/x quarters.  chunks 1-3 are PSUM-
                # gated at the boundary: run ALL k/v matmuls first so the 4
                # ps_q banks are not needed until ~7us into the chunk,
                # absorbing the previous chunk's attention-bank drain.
                xqs = []
                for db in range(4):
                    if qc == 0 and db < len(pre_xt):
                        xqs.append(pre_xt[db])
                    else:
                        xqs.append(xt_qload(qc, db))

                def xview(d):
                    db = d // DQ
                    return xqs[db][:, (d - db * DQ) * CH:(d - db * DQ + 1) * CH]

                def kv_d(d):
                    dsl = slice(d * HD, (d + 1) * HD)
                    st, sp = (d == 0), (d == DT - 1)
                    nc.tensor.matmul(ps_k[:], wk_sb[:, dsl], xview(d), start=st, stop=sp)
                    nc.tensor.matmul(ps_v[:], wv_sb[:, dsl], xview(d), start=st, stop=sp)

                def q_d(d):
                    st, sp = (d == 0), (d == DT - 1)
                    for hl in range(HPC):
                        nc.tensor.matmul(
                            ps_q[hl][:], wq_slice(d, hl), xview(d),
                            start=st, stop=sp,
                        )

                if qc == 0:
                    for db in range(4):
                        for d in range(db * DQ, (db + 1) * DQ):
                            kv_d(d)
                        for d in range(db * DQ, (db + 1) * DQ):
                            q_d(d)
                else:
                    for d in range(DT):
                        kv_d(d)
                    for d in range(DT):
                        q_d(d)

                # per-chunk slices of the RoPE tables: chunk 0's rope only
                # needs the first 512 columns, so don't front-load 4MB
                # constants and next-phase weights go on the scalar-engine
                # HWDGE queue so the SP queue stays dedicated to xt tiles
                nc.scalar.dma_start(cos_sb[:, csl], cosf[:, csl])
                nc.scalar.dma_start(sin_sb[:, csl], sinf[:, csl])
                if qc == 0:
                    nc.scalar.dma_start(ones_sb[:], onesm[:])
                    nc.scalar.dma_start(id_sb[:], ident[:])
                    nc.scalar.dma_start(
                        dpat_sb[:].rearrange("p (n c) -> p n c", c=CH),
                        dpat[:].rearrange("n p c -> p n c"),
                    )
                    for hl in range(HPC):
                        nc.scalar.dma_start(
                            wo_sb[hl][:], wos[hl * HD:(hl + 1) * HD, :],
                        )

                # v first: its PSUM->SBUF copy + transposes + vsb copies gate
                # the attention AV matmuls, and must not queue behind the
                # rope's scalar-engine staging copies
                vt = vtsbp.tile([P, CH], BF16, tag="vt", name=f"vt{qc}")
                nc.scalar.activation(vt[:], ps_v[:], AF.Copy)
                for jl in range(CH // P):
                    tps = psp.tile([P, P], BF16, tag="ps", name=f"tps{qc}_{jl}")
                    nc.tensor.transpose(tps[:], vt[:, jl * P:(jl + 1) * P], id_sb[:])
                    nc.scalar.activation(vsb[qc * (CH // P) + jl][:], tps[:], AF.Copy)

                # RoPE q heads -> per-chunk qT tiles; k -> resident kT
                qT = []
                for hl in range(HPC):
                    dst = qTp.tile([HD, CH], BF16, tag="qT", name=f"qT{qc}_{hl}")
                    qT.append(dst)
                # q heads first: attention on this chunk is gated by qT (old kT
                # columns are already rope'd); the new kT columns are only
                # needed at the diagonal j-tiles, late in the j-loop.
                rope_jobs = [(ps_q[hl], qT[hl][:]) for hl in range(HPC)]
                rope_jobs.append((ps_k, kT[:, csl]))
                if qc == 0:
                    rope_jobs.insert(0, rope_jobs.pop())
                for src, dst in rope_jobs:
                    # stage PSUM f32 -> SBUF bf16 on the scalar engine so
                    # every DVE operand is 2-byte + SBUF (4x DVE rate)
                    qf = tmpp.tile([P, CH], BF16, tag="tmp", name=f"qf{qc}")
                    nc.scalar.activation(qf[:], src[:], AF.Copy)
                    swp = tmpp.tile([P, CH], BF16, tag="tmp", name=f"sw{qc}")
                    nc.vector.stream_shuffle(swp[:], qf[:], shuffle_mask)
                    tcos = tmpp.tile([P, CH], BF16, tag="tmp", name=f"tc{qc}")
                    nc.vector.tensor_mul(tcos[:], qf[:], cos_sb[:, csl])
                    tsin = tmpp.tile([P, CH], BF16, tag="tmp", name=f"ts{qc}")
                    nc.vector.tensor_mul(tsin[:], swp[:], sin_sb[:, csl])
                    nc.vector.tensor_add(dst, tcos[:], tsin[:])

                if future:
                    saved_qT.append(qT)
                    continue

                # previous chunk's output projection: PE work that overlaps the
                # RoPE (DVE) of this chunk
                if prev_ohT is not None:
                    emit_oproj(qc - 1, prev_ohT)
                prev_ohT = emit_attn(qc, qT)

            if future:
                for qc in range(NQC):
                    emit_oproj(qc, emit_attn(qc, saved_qT[qc]))
            else:
                emit_oproj(NQC - 1, prev_ohT)

    nc.finalize()
    return nc


def _get_program(sched, n_pat):
    key = (sched, n_pat)
    if key not in _prog_cache:
        _prog_cache[key] = _build_program(sched, n_pat)
    return _prog_cache[key]


def kernel(x, wq, wk, wv, wo, freqs_cos, freqs_sin, mask, start_pos=0, **_kw):
    from concourse.bass_utils import run_bass_kernel_spmd
    import ml_dtypes

    BF = ml_dtypes.bfloat16
    x = np.asarray(x, dtype=np.float32)
    wq = np.asarray(wq, dtype=np.float32)
    wk = np.asarray(wk, dtype=np.float32)
    wv = np.asarray(wv, dtype=np.float32)
    wo = np.asarray(wo, dtype=np.float32)
    fc = np.asarray(freqs_cos, dtype=np.float32)
    fs = np.asarray(freqs_sin, dtype=np.float32)

    sched, plist = _mask_schedule(mask)
    nc = _get_program(sched, len(plist))

    # RoPE tables expanded to head-dim channels (sin sign-interleaved so the
    # pair-swap shuffle needs no negation).
    cosf = np.repeat(fc.T, 2, axis=0)                            # [HD, S]
    sinf = np.repeat(fs.T, 2, axis=0)
    sinf[0::2, :] *= -1.0
    cosf = np.ascontiguousarray(cosf.astype(BF))
    sinf = np.ascontiguousarray(sinf.astype(BF))

    onesm = np.ones((P, P), dtype=BF)
    ident = np.eye(P, dtype=BF)
    dpat_arr = (
        np.stack(plist, axis=0).astype(BF)
        if plist
        else np.zeros((1, P, CH), dtype=BF)
    )

    xtb = [np.ascontiguousarray(x[b].T.astype(BF)) for b in range(B)]
    wq_bf = wq.astype(BF)
    wk_bf = wk.astype(BF)
    wv_bf = wv.astype(BF)
    in_maps = []
    for c in range(N_CORES):
        b, g = divmod(c, 4)
        hcols = np.concatenate(
            [np.arange(h * HD, (h + 1) * HD) for h in (g, g + 4, g + 8, g + 12)]
        )
        in_maps.append(
            dict(
                xt=xtb[b],
                wqs=np.ascontiguousarray(wq_bf[:, hcols]),
                wks=np.ascontiguousarray(wk_bf[:, g * HD:(g + 1) * HD]),
                wvs=np.ascontiguousarray(wv_bf[:, g * HD:(g + 1) * HD]),
                wos=np.ascontiguousarray(wo[hcols, :].astype(BF)),
                cosf=cosf,
                sinf=sinf,
                onesm=onesm,
                ident=ident,
                dpat=dpat_arr,
            )
        )

    res = run_bass_kernel_spmd(nc, in_maps, list(range(N_CORES)))

    out_full = np.empty((B, S, DIM), dtype=np.float32)
    nit = CH // P
    for c in range(N_CORES):
        b, g = divmod(c, 4)
        o = np.asarray(res.results[c]["out"], dtype=np.float32)
        for qc in range(NQC):
            r0 = qc * CH + g * P
            out_full[b, r0:r0 + P, :] = o[qc * nit:(qc + 1) * nit].reshape(P, DIM)
    return out_full

